# revision 18
# baseline (speedup 1.0000x reference)
"""BoT tokenizer kernel for Trainium2 (Bass/Tile), 8-core data parallel.

All 25 output tokens are computed on the TensorEngine as bf16 matmuls with
an exact fp32 -> 3x bf16 mantissa split (8+8+8 = 24 bits):

    x = a0 + a1 + a2 (each bf16, split exact by construction)
    x*w = sum_{i,j} ai*wj   (each bf16 product is exact in fp32)

 - single-feature token k: K=12 matmul (9 cross products + 3 bias rows
   against a ones column)
 - fore token: 9 features -> K = 9*9+3 = 84
 - palm token: 7 features -> K = 7*9+3 = 66

bf16 matmuls stream 1 col/cycle (vs 4 for fp32), so the PE produces each
[128,512] token tile in ~215ns. PSUM->SBUF copies are split between
VectorE and ScalarE. The kernel is then purely output-DMA bound:
each core writes 1024*25*512*4 = 52.4 MB of fp32 to HBM.
"""

import numpy as np

FORE_IDX = [0, 1, 2, 27, 28, 32, 33, 34, 38]
PALM_IDX = [4, 29, 30, 31, 35, 36, 37]
SINGLE_IDX = [3] + list(range(5, 27))

B = 8192
D = 512
T = 25
N_CORES = 8
B_LOC = B // N_CORES          # 1024 rows per core
CHUNK = 128
N_CHUNKS = B_LOC // CHUNK     # 8
ROW = T * D                   # 12800
NS = 23

# token id for single sensor k: k=0 -> token 1 (wrist), k>=1 -> token k+2
TOK_OF_SINGLE = [1] + list(range(3, 25))
# out-tile token groups for finer DMA pipelining
GROUPS = [(0, 4), (4, 8), (8, 12), (12, 16), (16, 20), (20, 25)]

KF = 9 * 9 + 3                # 84
KP = 7 * 9 + 3                # 66
KS = 12
# singles packed 3 per tile at 32-partition offsets (matmul base partition
# must be 0/32/64)
S_TILES = [(a, min(a + 3, NS)) for a in range(0, NS, 3)]
S_STRIDE = 32

_prog_cache = {}


def _k_of_tok(t):
    return 0 if t == 1 else t - 2


def _build_program():
    import concourse.bacc as bacc
    import concourse.mybir as mybir
    import concourse.tile as tile
    from concourse.bass import ts

    f32 = mybir.dt.float32
    bf16 = mybir.dt.bfloat16

    nc = bacc.Bacc("TRN2", target_bir_lowering=False, debug=False,
                   num_devices=N_CORES)

    lf_d = nc.dram_tensor("lf", [KF, B_LOC], bf16, kind="ExternalInput")
    lp_d = nc.dram_tensor("lp", [KP, B_LOC], bf16, kind="ExternalInput")
    rf_d = nc.dram_tensor("rf", [KF, D], bf16, kind="ExternalInput")
    rp_d = nc.dram_tensor("rp", [KP, D], bf16, kind="ExternalInput")
    ls_d = [nc.dram_tensor(f"ls{i}", [(b - a) * S_STRIDE, B_LOC], bf16,
                           kind="ExternalInput")
            for i, (a, b) in enumerate(S_TILES)]
    rs_d = [nc.dram_tensor(f"rs{i}", [(b - a) * S_STRIDE, D], bf16,
                           kind="ExternalInput")
            for i, (a, b) in enumerate(S_TILES)]
    out_d = nc.dram_tensor("out", [B_LOC, ROW], f32, kind="ExternalOutput")

    with tile.TileContext(nc) as tc:
        with (
            tc.tile_pool(name="cst", bufs=1) as cst,
            tc.tile_pool(name="op", bufs=3) as op,
            tc.tile_pool(name="pp", bufs=7, space="PSUM") as pp,
        ):
            lf_s = cst.tile([KF, B_LOC], bf16)
            nc.sync.dma_start(out=lf_s[:], in_=lf_d[:])
            lp_s = cst.tile([KP, B_LOC], bf16)
            nc.sync.dma_start(out=lp_s[:], in_=lp_d[:])
            rf_s = cst.tile([KF, D], bf16)
            nc.sync.dma_start(out=rf_s[:], in_=rf_d[:])
            rp_s = cst.tile([KP, D], bf16)
            nc.sync.dma_start(out=rp_s[:], in_=rp_d[:])
            ls_s, rs_s = [], []
            for i, (a, b) in enumerate(S_TILES):
                lt = cst.tile([(b - a) * S_STRIDE, B_LOC], bf16,
                              name=f"ls{i}_s")
                nc.sync.dma_start(out=lt[:], in_=ls_d[i][:])
                ls_s.append(lt)
                rt = cst.tile([(b - a) * S_STRIDE, D], bf16, name=f"rs{i}_s")
                nc.sync.dma_start(out=rt[:], in_=rs_d[i][:])
                rs_s.append(rt)

            # PE warm-up: ~10 back-to-back dummy matmuls while the input
            # DMAs land, so HAM is at full clock when real work starts
            wl = cst.tile([2, CHUNK], bf16)
            wr = cst.tile([2, D], bf16)
            nc.gpsimd.memset(wl[:], 0)
            nc.gpsimd.memset(wr[:], 0)
            wp_t = pp.tile([CHUNK, D], f32, tag="warm", bufs=1)
            for _ in range(10):
                nc.tensor.matmul(wp_t[:], wl[:], wr[:], start=True, stop=True)

            for c in range(N_CHUNKS):
                ncopy = 0
                for gi, (t0, t1) in enumerate(GROUPS):
                    o_t = op.tile([CHUNK, (t1 - t0) * D], f32, tag=f"out{gi}")
                    for t in range(t0, t1):
                        dst = o_t[:, ts(t - t0, D)]
                        if t == 0:
                            lhsT = lf_s[:, ts(c, CHUNK)]
                            rhs = rf_s[:]
                        elif t == 2:
                            lhsT = lp_s[:, ts(c, CHUNK)]
                            rhs = rp_s[:]
                        else:
                            k = _k_of_tok(t)
                            i = k // 3
                            off = S_STRIDE * (k - S_TILES[i][0])
                            lhsT = ls_s[i][off:off + KS, ts(c, CHUNK)]
                            rhs = rs_s[i][off:off + KS, :]
                        p_t = pp.tile([CHUNK, D], f32)
                        nc.tensor.matmul(p_t[:], lhsT, rhs,
                                         start=True, stop=True)
                        if ncopy % 2 == 0:
                            nc.vector.tensor_copy(dst, p_t[:])
                        else:
                            nc.scalar.copy(dst, p_t[:])
                        ncopy += 1
                    nc.sync.dma_start(
                        out=out_d[ts(c, CHUNK), t0 * D:t1 * D], in_=o_t[:])

    nc.compile()
    return nc


def _split3(v):
    """Exact fp32 -> (bf16, bf16, bf16) mantissa split: v = s0+s1+s2."""
    import ml_dtypes
    bf = ml_dtypes.bfloat16
    v = np.asarray(v, np.float32)
    s0 = v.astype(bf)
    r1 = v - s0.astype(np.float32)
    s1 = r1.astype(bf)
    r2 = r1 - s1.astype(np.float32)
    s2 = r2.astype(bf)
    return s0, s1, s2


def _lhs_rows(xcols):
    """lhsT rows for a feature block: a0,a0,a0,a1,a1,a1,a2,a2,a2 per feat.

    xcols: [B, F] fp32 -> [9F, B] bf16"""
    import ml_dtypes
    Bn, F = xcols.shape
    s0, s1, s2 = _split3(xcols)          # each [B, F]
    out = np.empty((F, 9, Bn), dtype=ml_dtypes.bfloat16)
    for i, s in enumerate((s0, s1, s2)):
        out[:, 3 * i:3 * i + 3, :] = s.T[:, None, :]
    return out.reshape(9 * F, Bn)


def _rhs_rows(wcols):
    """rhs rows for a feature block: w0,w1,w2,w0,w1,w2,w0,w1,w2 per feat.

    wcols: [F, D] fp32 -> [9F, D] bf16"""
    import ml_dtypes
    F, Dn = wcols.shape
    s0, s1, s2 = _split3(wcols)
    out = np.empty((F, 3, 3, Dn), dtype=ml_dtypes.bfloat16)
    for j, s in enumerate((s0, s1, s2)):
        out[:, :, j, :] = s[:, None, :]
    return out.reshape(9 * F, Dn)


def _host_prep(x, Wf, bf_, Wp, bp, Ws, bs):
    import ml_dtypes
    bf16 = ml_dtypes.bfloat16

    ones3 = np.ones((3, B), dtype=bf16)

    def bias_rows(bias):
        b0, b1, b2 = _split3(bias)       # [D] each
        return np.stack([b0, b1, b2])    # [3, D]

    # fore: lhsT [84, B], rhs [84, D]
    lf = np.concatenate([_lhs_rows(x[:, FORE_IDX]), ones3])
    rf = np.concatenate([_rhs_rows(np.asarray(Wf.T)), bias_rows(bf_)])
    # palm: [66, *]
    lp = np.concatenate([_lhs_rows(x[:, PALM_IDX]), ones3])
    rp = np.concatenate([_rhs_rows(np.asarray(Wp.T)), bias_rows(bp)])

    # singles: per sensor a [12, *] block, padded to 32-partition slots
    ls_all = np.zeros((NS * S_STRIDE, B), dtype=bf16)
    rs_all = np.zeros((NS * S_STRIDE, D), dtype=bf16)
    xs = x[:, SINGLE_IDX]                # [B, 23]
    for k in range(NS):
        o = S_STRIDE * k
        ls_all[o:o + 9] = _lhs_rows(xs[:, k:k + 1])
        ls_all[o + 9:o + KS] = ones3
        rs_all[o:o + 9] = _rhs_rows(Ws[k:k + 1])
        rs_all[o + 9:o + KS] = bias_rows(bs[k])
    return lf, rf, lp, rp, ls_all, rs_all


def kernel(x, Wf, bf, Wp, bp, Ws, bs, _trace=False, _spmd_kwargs=None):
    from concourse.bass_utils import run_bass_kernel_spmd

    x = np.asarray(x, np.float32)
    lf, rf, lp, rp, ls_all, rs_all = _host_prep(
        x, np.asarray(Wf, np.float32), np.asarray(bf, np.float32),
        np.asarray(Wp, np.float32), np.asarray(bp, np.float32),
        np.asarray(Ws, np.float32), np.asarray(bs, np.float32))

    if "nc" not in _prog_cache:
        _prog_cache["nc"] = _build_program()
    nc = _prog_cache["nc"]

    in_maps = []
    for i in range(N_CORES):
        sl = slice(i * B_LOC, (i + 1) * B_LOC)
        m = {
            "lf": np.ascontiguousarray(lf[:, sl]),
            "lp": np.ascontiguousarray(lp[:, sl]),
            "rf": rf,
            "rp": rp,
        }
        for j, (a, b) in enumerate(S_TILES):
            m[f"ls{j}"] = np.ascontiguousarray(
                ls_all[S_STRIDE * a:S_STRIDE * b, sl])
            m[f"rs{j}"] = np.ascontiguousarray(rs_all[S_STRIDE * a:S_STRIDE * b])
        in_maps.append(m)

    kwargs = dict(_spmd_kwargs or {})
    res = run_bass_kernel_spmd(nc, in_maps, core_ids=list(range(N_CORES)),
                               trace=_trace, **kwargs)
    out = np.concatenate([r["out"] for r in res.results], axis=0)
    if _trace:
        kernel.last_results = res
    return out.reshape(B, T, D)


# revision 19
# speedup vs baseline: 1.1800x; 1.1800x over previous
"""BoT tokenizer kernel for Trainium2 (Bass/Tile), 8-core data parallel.

All 25 output tokens are computed on the TensorEngine as bf16 matmuls with
an exact fp32 -> 3x bf16 mantissa split (8+8+8 = 24 bits):

    x = a0 + a1 + a2 (each bf16, split exact by construction)
    x*w = sum_{i,j} ai*wj   (each bf16 product is exact in fp32)

 - single-feature token k: K=12 matmul (9 cross products + 3 bias rows
   against a ones column)
 - fore token: 9 features -> K = 9*9+3 = 84
 - palm token: 7 features -> K = 7*9+3 = 66

bf16 matmuls stream 1 col/cycle (vs 4 for fp32), so the PE produces each
[128,512] token tile in ~215ns. PSUM->SBUF copies are split between
VectorE and ScalarE. The kernel is then purely output-DMA bound:
each core writes 1024*25*512*4 = 52.4 MB of fp32 to HBM.
"""

import numpy as np

FORE_IDX = [0, 1, 2, 27, 28, 32, 33, 34, 38]
PALM_IDX = [4, 29, 30, 31, 35, 36, 37]
SINGLE_IDX = [3] + list(range(5, 27))

B = 8192
D = 512
T = 25
N_CORES = 8
B_LOC = B // N_CORES          # 1024 rows per core
CHUNK = 128
N_CHUNKS = B_LOC // CHUNK     # 8
ROW = T * D                   # 12800
NS = 23

# token id for single sensor k: k=0 -> token 1 (wrist), k>=1 -> token k+2
TOK_OF_SINGLE = [1] + list(range(3, 25))
# out-tile token groups for finer DMA pipelining
GROUPS = [(0, 6), (6, 12), (12, 19), (19, 25)]

KF = 9 * 9 + 3                # 84
KP = 7 * 9 + 3                # 66
KS = 12
# singles packed 3 per tile at 32-partition offsets (matmul base partition
# must be 0/32/64)
S_TILES = [(a, min(a + 3, NS)) for a in range(0, NS, 3)]
S_STRIDE = 32

_prog_cache = {}


def _k_of_tok(t):
    return 0 if t == 1 else t - 2


def _build_program():
    import concourse.bacc as bacc
    import concourse.mybir as mybir
    import concourse.tile as tile
    from concourse.bass import ts

    f32 = mybir.dt.float32
    bf16 = mybir.dt.bfloat16

    nc = bacc.Bacc("TRN2", target_bir_lowering=False, debug=False,
                   num_devices=N_CORES)

    lf_d = nc.dram_tensor("lf", [KF, B_LOC], bf16, kind="ExternalInput")
    lp_d = nc.dram_tensor("lp", [KP, B_LOC], bf16, kind="ExternalInput")
    rf_d = nc.dram_tensor("rf", [KF, D], bf16, kind="ExternalInput")
    rp_d = nc.dram_tensor("rp", [KP, D], bf16, kind="ExternalInput")
    ls_d = [nc.dram_tensor(f"ls{i}", [(b - a) * S_STRIDE, B_LOC], bf16,
                           kind="ExternalInput")
            for i, (a, b) in enumerate(S_TILES)]
    rs_d = [nc.dram_tensor(f"rs{i}", [(b - a) * S_STRIDE, D], bf16,
                           kind="ExternalInput")
            for i, (a, b) in enumerate(S_TILES)]
    out_d = nc.dram_tensor("out", [B_LOC, ROW], f32, kind="ExternalOutput")

    with tile.TileContext(nc) as tc:
        with (
            tc.tile_pool(name="cst", bufs=1) as cst,
            tc.tile_pool(name="op", bufs=3) as op,
            tc.tile_pool(name="pp", bufs=7, space="PSUM") as pp,
        ):
            lf_s = cst.tile([KF, B_LOC], bf16)
            nc.sync.dma_start(out=lf_s[:], in_=lf_d[:])
            lp_s = cst.tile([KP, B_LOC], bf16)
            nc.sync.dma_start(out=lp_s[:], in_=lp_d[:])
            rf_s = cst.tile([KF, D], bf16)
            nc.sync.dma_start(out=rf_s[:], in_=rf_d[:])
            rp_s = cst.tile([KP, D], bf16)
            nc.sync.dma_start(out=rp_s[:], in_=rp_d[:])
            ls_s, rs_s = [], []
            for i, (a, b) in enumerate(S_TILES):
                lt = cst.tile([(b - a) * S_STRIDE, B_LOC], bf16,
                              name=f"ls{i}_s")
                nc.sync.dma_start(out=lt[:], in_=ls_d[i][:])
                ls_s.append(lt)
                rt = cst.tile([(b - a) * S_STRIDE, D], bf16, name=f"rs{i}_s")
                nc.sync.dma_start(out=rt[:], in_=rs_d[i][:])
                rs_s.append(rt)

            # PE warm-up: ~10 back-to-back dummy matmuls while the input
            # DMAs land, so HAM is at full clock when real work starts
            wl = cst.tile([2, CHUNK], bf16)
            wr = cst.tile([2, D], bf16)
            nc.gpsimd.memset(wl[:], 0)
            nc.gpsimd.memset(wr[:], 0)
            wp_t = pp.tile([CHUNK, D], f32, tag="warm", bufs=1)
            for _ in range(10):
                nc.tensor.matmul(wp_t[:], wl[:], wr[:], start=True, stop=True)

            for c in range(N_CHUNKS):
                ncopy = 0
                for gi, (t0, t1) in enumerate(GROUPS):
                    o_t = op.tile([CHUNK, (t1 - t0) * D], f32, tag=f"out{gi}")
                    for t in range(t0, t1):
                        dst = o_t[:, ts(t - t0, D)]
                        if t == 0:
                            lhsT = lf_s[:, ts(c, CHUNK)]
                            rhs = rf_s[:]
                        elif t == 2:
                            lhsT = lp_s[:, ts(c, CHUNK)]
                            rhs = rp_s[:]
                        else:
                            k = _k_of_tok(t)
                            i = k // 3
                            off = S_STRIDE * (k - S_TILES[i][0])
                            lhsT = ls_s[i][off:off + KS, ts(c, CHUNK)]
                            rhs = rs_s[i][off:off + KS, :]
                        p_t = pp.tile([CHUNK, D], f32)
                        nc.tensor.matmul(p_t[:], lhsT, rhs,
                                         start=True, stop=True)
                        if ncopy % 2 == 0:
                            nc.vector.tensor_copy(dst, p_t[:])
                        else:
                            nc.scalar.copy(dst, p_t[:])
                        ncopy += 1
                    nc.sync.dma_start(
                        out=out_d[ts(c, CHUNK), t0 * D:t1 * D], in_=o_t[:])

    nc.compile()
    return nc


def _split3(v):
    """Exact fp32 -> (bf16, bf16, bf16) mantissa split: v = s0+s1+s2."""
    import ml_dtypes
    bf = ml_dtypes.bfloat16
    v = np.asarray(v, np.float32)
    s0 = v.astype(bf)
    r1 = v - s0.astype(np.float32)
    s1 = r1.astype(bf)
    r2 = r1 - s1.astype(np.float32)
    s2 = r2.astype(bf)
    return s0, s1, s2


def _lhs_rows(xcols):
    """lhsT rows for a feature block: a0,a0,a0,a1,a1,a1,a2,a2,a2 per feat.

    xcols: [B, F] fp32 -> [9F, B] bf16"""
    import ml_dtypes
    Bn, F = xcols.shape
    s0, s1, s2 = _split3(xcols)          # each [B, F]
    out = np.empty((F, 9, Bn), dtype=ml_dtypes.bfloat16)
    for i, s in enumerate((s0, s1, s2)):
        out[:, 3 * i:3 * i + 3, :] = s.T[:, None, :]
    return out.reshape(9 * F, Bn)


def _rhs_rows(wcols):
    """rhs rows for a feature block: w0,w1,w2,w0,w1,w2,w0,w1,w2 per feat.

    wcols: [F, D] fp32 -> [9F, D] bf16"""
    import ml_dtypes
    F, Dn = wcols.shape
    s0, s1, s2 = _split3(wcols)
    out = np.empty((F, 3, 3, Dn), dtype=ml_dtypes.bfloat16)
    for j, s in enumerate((s0, s1, s2)):
        out[:, :, j, :] = s[:, None, :]
    return out.reshape(9 * F, Dn)


def _host_prep(x, Wf, bf_, Wp, bp, Ws, bs):
    import ml_dtypes
    bf16 = ml_dtypes.bfloat16

    ones3 = np.ones((3, B), dtype=bf16)

    def bias_rows(bias):
        b0, b1, b2 = _split3(bias)       # [D] each
        return np.stack([b0, b1, b2])    # [3, D]

    # fore: lhsT [84, B], rhs [84, D]
    lf = np.concatenate([_lhs_rows(x[:, FORE_IDX]), ones3])
    rf = np.concatenate([_rhs_rows(np.asarray(Wf.T)), bias_rows(bf_)])
    # palm: [66, *]
    lp = np.concatenate([_lhs_rows(x[:, PALM_IDX]), ones3])
    rp = np.concatenate([_rhs_rows(np.asarray(Wp.T)), bias_rows(bp)])

    # singles: per sensor a [12, *] block, padded to 32-partition slots
    ls_all = np.zeros((NS * S_STRIDE, B), dtype=bf16)
    rs_all = np.zeros((NS * S_STRIDE, D), dtype=bf16)
    xs = x[:, SINGLE_IDX]                # [B, 23]
    for k in range(NS):
        o = S_STRIDE * k
        ls_all[o:o + 9] = _lhs_rows(xs[:, k:k + 1])
        ls_all[o + 9:o + KS] = ones3
        rs_all[o:o + 9] = _rhs_rows(Ws[k:k + 1])
        rs_all[o + 9:o + KS] = bias_rows(bs[k])
    return lf, rf, lp, rp, ls_all, rs_all


def kernel(x, Wf, bf, Wp, bp, Ws, bs, _trace=False, _spmd_kwargs=None):
    from concourse.bass_utils import run_bass_kernel_spmd

    x = np.asarray(x, np.float32)
    lf, rf, lp, rp, ls_all, rs_all = _host_prep(
        x, np.asarray(Wf, np.float32), np.asarray(bf, np.float32),
        np.asarray(Wp, np.float32), np.asarray(bp, np.float32),
        np.asarray(Ws, np.float32), np.asarray(bs, np.float32))

    if "nc" not in _prog_cache:
        _prog_cache["nc"] = _build_program()
    nc = _prog_cache["nc"]

    in_maps = []
    for i in range(N_CORES):
        sl = slice(i * B_LOC, (i + 1) * B_LOC)
        m = {
            "lf": np.ascontiguousarray(lf[:, sl]),
            "lp": np.ascontiguousarray(lp[:, sl]),
            "rf": rf,
            "rp": rp,
        }
        for j, (a, b) in enumerate(S_TILES):
            m[f"ls{j}"] = np.ascontiguousarray(
                ls_all[S_STRIDE * a:S_STRIDE * b, sl])
            m[f"rs{j}"] = np.ascontiguousarray(rs_all[S_STRIDE * a:S_STRIDE * b])
        in_maps.append(m)

    kwargs = dict(_spmd_kwargs or {})
    res = run_bass_kernel_spmd(nc, in_maps, core_ids=list(range(N_CORES)),
                               trace=_trace, **kwargs)
    out = np.concatenate([r["out"] for r in res.results], axis=0)
    if _trace:
        kernel.last_results = res
    return out.reshape(B, T, D)
